# revision 37
# baseline (speedup 1.0000x reference)
"""Trainium2 Bass kernel for the 2-qubit EstimatorQNN forward pass.

The circuit collapses analytically to THREE cosines (out = u^T K v in the
basis u=(1,cos pi x0,sin pi x0), v=(1,cos pi x1,sin pi x1); K row 0 is zero
and the 2x2 trailing block is rank-1 — both asserted at derive time):

  out = Rk*cos(pi x0 - phk) + Rw*Rz*cos(pi x0 - phw)*cos(pi x1 - phz)

Measured engine realities on TRN2 drive the design (HW traces):
ACT ~0.93ns/elem any activation (no fp16 speedup), Sin table valid on
[-pi,pi] ONLY, one-time 1.3us Sin table load; DVE fp16 tensor_scalar
0.32ns/elem (4x), tensor_tensor 0.58 (2x), stt/custom-DVE 1.1 (1x);
Pool ~2.05ns/elem, cross-tile ADD pathological (~14ns/elem); the SP
sequencer spends ~600ns per DMA and STALLS the whole queue at a waiting
out-DMA, so input DMAs must all issue first.

Per-element math (device ships TWO interleaved streams, host adds them):
  t0  = sin(pi(x0+e0+m0))            m0 in {0,-+2}: one-sided range wrap
  t1  = sin(pi(x0+e1+m1))            via is_gt/is_lt mask (DVE ts) + add
  t2' = sin((pi/2)x1 + psi)          half-angle, in-range, NO wrap
  M2  = t1*(1-2*t2'^2)               = t1*cos(pi x1 - phz)
  host: out = S*(ca*M2 + cb*t0)      (ca,cb) from the amplitude ratio

The phase shifts e_j ride in the per-slot Sin bias APs (f32), so the DVE
wrap only adds the {0,-+2} mask.  Per tile (H=1024 per partition-row):
  SP  : one input DMA per tile, ALL issued up front (input fits SBUF);
        one output DMA per tile at the end of the queue
  ACT : sin(t2') first (it only needs the DMA), then sin(t1), then
        sin(t0) written DIRECTLY into the out slab next to M2
  DVE : m0,m1 (ts) w0,w1 (tt) G=t2'^2 (tt) V=1-2G (ts) M2=t1*V (tt)
Pool stays out of the main loop (its ~2ns/elem ops plus the semaphore
glue measured slower than keeping everything on DVE), and every tile
pool uses bufs=NT so no buffer is ever reused (no WAR semaphores).
A tiny warm-up Sin overlaps the ACT table load with the input DMAs.
fp16 everywhere in SBUF and over DMA (in 2.1MB + out 2.1MB per core).

Only (e0, e1, psi) are baked into the program; amplitude ratio p, scale S
and the case live in the host combine, so one compiled program serves any
weights with the same phases (and the neuronxcc disk cache makes repeat
compiles for the same weights instant).
"""

import sys

if "/opt/trn_rl_repo" not in sys.path:
    sys.path.insert(0, "/opt/trn_rl_repo")

import numpy as np

import concourse.bass as bass
import concourse.bacc as bacc
import concourse.mybir as mybir
import concourse.tile as tile
from concourse.bass_utils import run_bass_kernel_spmd

N_CORES = 8
B = 4194304
BC = B // N_CORES            # samples per core
P = 128                      # SBUF partitions
H = 1024                     # samples per partition-row per tile
NT = BC // (P * H)           # 4 tiles

F16 = mybir.dt.float16
F32 = mybir.dt.float32
PI = float(np.float64(np.pi))

_N_QUBITS, _N_LAYERS = 2, 2


# ----------------------------------------------------------------- host math

def _circuit_unitary(w):
    """Fixed 4x4 unitary of the variational layers (float64 complex)."""
    def rx(t):
        c, s = np.cos(t / 2), np.sin(t / 2)
        return np.array([[c, -1j * s], [-1j * s, c]])

    def rz(t):
        c, s = np.cos(t / 2), np.sin(t / 2)
        return np.array([[c - 1j * s, 0], [0, c + 1j * s]])

    def ry(t):
        c, s = np.cos(t / 2), np.sin(t / 2)
        return np.array([[c, -s], [s, c]])

    I2 = np.eye(2)
    CNOT = np.array(
        [[1, 0, 0, 0], [0, 1, 0, 0], [0, 0, 0, 1], [0, 0, 1, 0]], dtype=complex
    )
    U = np.eye(4, dtype=complex)
    off = 0
    for _ in range(_N_LAYERS):
        for q in range(_N_QUBITS):
            for G in (
                rx(w[off + q * 3 + 0]),
                rz(w[off + q * 3 + 1]),
                ry(w[off + q * 3 + 2]),
            ):
                M = np.kron(G, I2) if q == 0 else np.kron(I2, G)
                U = M @ U
        U = CNOT @ U
        off += _N_QUBITS * 3
    return U


def _derive_consts(weights):
    """weights[12] -> (e0, e1, psi, p, case_b, S).

    out = Rk*cos(pi x0 - phk) + Rw*Rz*cos(pi x0 - phw)*cos(pi x1 - phz)
    e0, e1 in (-1/2, 3/2]: sin-form shifts for the two x0 cosines.
    psi = -phz/2: half-angle bias for the x1 cosine.
    case_b = (Rw*Rz >= Rk); p = min/max of the amplitudes; S = max.
    """
    w = np.asarray(weights, dtype=np.float64)
    U = _circuit_unitary(w)
    Z0 = np.diag([1.0, 1.0, -1.0, -1.0])
    A = np.real(U.conj().T @ Z0 @ U)

    I2 = np.eye(2)
    Z = np.diag([1.0, -1.0])
    X = np.array([[0.0, 1.0], [1.0, 0.0]])
    Pb = [I2, Z, X]
    K = np.zeros((3, 3))
    for p_ in range(3):
        for q_ in range(3):
            K[p_, q_] = 0.25 * sum(
                A[2 * i + j, 2 * k + l] * Pb[p_][i, k] * Pb[q_][j, l]
                for i in range(2)
                for j in range(2)
                for k in range(2)
                for l in range(2)
            )

    scale = max(np.abs(K).max(), 1e-30)
    assert np.abs(K[0]).max() < 1e-9 * scale, (
        f"structure violated: K row0 nonzero ({K[0]})"
    )

    K10, K20 = K[1, 0], K[2, 0]
    M = K[1:, 1:]
    u_, s_, vt_ = np.linalg.svd(M)
    assert s_[1] < 1e-9 * scale, f"structure violated: rank-1 residual {s_}"
    wvec = u_[:, 0] * s_[0]
    zvec = vt_[0, :]

    Rk, phk = float(np.hypot(K10, K20)), float(np.arctan2(K20, K10))
    Rw, phw = float(np.hypot(*wvec)), float(np.arctan2(wvec[1], wvec[0]))
    Rz, phz = float(np.hypot(*zvec)), float(np.arctan2(zvec[1], zvec[0]))
    Rwz = Rw * Rz

    def efold(phi):
        # cos(pi x - phi) = sin(pi(x + e)), e = 1/2 - phi/pi, d in (-1,1]
        d = -phi / np.pi
        d = d - 2 * np.floor((d + 1) / 2)  # (-1, 1]
        return float(d + 0.5)              # (-1/2, 3/2]

    S = max(Rk, Rwz)
    if S < 1e-30:
        return (0.5, 0.5, 0.0, 0.0, True, 0.0)
    case_b = Rwz >= Rk
    p = (Rk / Rwz) if case_b else (Rwz / Rk)
    return (efold(phk), efold(phw), float(-phz / 2), float(p), bool(case_b),
            float(S))


# ------------------------------------------------------------- device program

def build_program(consts, nt=NT, h=H):
    """Per-core Bass program; only (e0, e1, psi) are baked in."""
    e0, e1, psi = (float(np.float32(v)) for v in consts[:3])
    hh = h // 2

    nc = bacc.Bacc("TRN2", target_bir_lowering=False, debug=False)

    # f32 per-slot Sin bias APs + a warm-up input so the 1.3us Sin table
    # load overlaps the first input DMA instead of the first real sin
    biases = []
    for i, bval in enumerate((PI * e0, PI * e1, psi)):
        t = nc.alloc_sbuf_tensor(f"const-b{i}", [P, 1], F32)
        nc.gpsimd.memset(t.ap(), bval)
        biases.append(t)
    warm = nc.alloc_sbuf_tensor("warm", [P, 1], F16)
    nc.gpsimd.memset(warm.ap(), 0.0)
    nc.all_engine_barrier()

    xin = nc.dram_tensor("xin", [nt, P, 2 * h], F16, kind="ExternalInput")
    # one out slab [M2 | t0] per tile; separate tensors avoid whole-tensor
    # WAW ordering between the out-DMAs (one DMA per tile measured faster
    # than per-stream DMAs: fewer SP configs)
    youts = [
        nc.dram_tensor(f"y{t}", [P, 2 * h], F16, kind="ExternalOutput")
        for t in range(nt)
    ]

    SIN = mybir.ActivationFunctionType.Sin
    MULT = mybir.AluOpType.mult
    ADD = mybir.AluOpType.add
    ISGT = mybir.AluOpType.is_gt
    ISLT = mybir.AluOpType.is_lt

    def mask_args(e):
        # one-sided wrap of x+e into [-1,1]: m in {0, -+2}
        if e > 0:
            return (float(np.float32(1.0 - e)), -2.0, ISGT)
        return (float(np.float32(-1.0 - e)), 2.0, ISLT)

    thr0, per0, cmp0 = mask_args(e0)
    thr1, per1, cmp1 = mask_args(e1)

    with tile.TileContext(nc) as tc:
        with (
            tc.tile_pool(name="xpool", bufs=nt) as xpool,
            tc.tile_pool(name="kpool", bufs=nt) as kpool,
            tc.tile_pool(name="wpool", bufs=nt) as wpool,
            tc.tile_pool(name="tpool", bufs=nt) as tpool,
            tc.tile_pool(name="gpool", bufs=nt) as gpool,
            tc.tile_pool(name="vpool", bufs=nt) as vpool,
            tc.tile_pool(name="opool", bufs=nt) as opool,
        ):
            # warm-up: loads the Sin activation table while DMAs run
            nc.scalar.activation(warm.ap(), warm.ap(), SIN, bias=0.0, scale=1.0)

            # ALL input DMAs first: a waiting out-DMA stalls the whole
            # in-order SP queue, so no out-DMA may precede an input DMA.
            # (Pool-issued DGE measured ~2.7us SLOWER to first compute.)
            # bufs=nt on every pool: no buffer reuse -> no WAR semaphores.
            Xs = []
            for t in range(nt):
                X = xpool.tile([P, 2 * h], F16, tag="x")
                nc.sync.dma_start(X[:], xin[t])
                Xs.append(X)

            tiles = {}

            def pre(t):
                X0, X1 = Xs[t][:, 0:h], Xs[t][:, h:2 * h]
                MK = kpool.tile([P, 2 * h], F16, tag="mk")
                W = wpool.tile([P, 2 * h], F16, tag="w")
                T = tpool.tile([P, 2 * h], F16, tag="t")  # [t2' | t1]
                # t2' first: it only needs the input DMA, so ACT starts
                # without waiting for any DVE work
                nc.scalar.activation(
                    T[:, 0:h], X1, SIN, bias=biases[2].ap(), scale=PI / 2
                )
                nc.vector.tensor_scalar(MK[:, 0:h], X0, thr0, per0, cmp0, MULT)
                nc.vector.tensor_scalar(MK[:, h:2 * h], X0, thr1, per1, cmp1, MULT)
                nc.vector.tensor_tensor(W[:, h:2 * h], X0, MK[:, h:2 * h], ADD)
                nc.vector.tensor_tensor(W[:, 0:h], X0, MK[:, 0:h], ADD)
                nc.scalar.activation(
                    T[:, h:2 * h], W[:, h:2 * h], SIN, bias=biases[1].ap(), scale=PI
                )
                tiles[t] = (T, W)

            def post(t):
                T, W = tiles.pop(t)
                t2, t1 = T[:, 0:h], T[:, h:2 * h]
                G = gpool.tile([P, h], F16, tag="g")
                V = vpool.tile([P, h], F16, tag="v")
                OT = opool.tile([P, 2 * h], F16, tag="ot")  # [M2 | t0]
                nc.vector.tensor_tensor(G[:], t2, t2, MULT)
                nc.vector.tensor_scalar(V[:], G[:], -2.0, 1.0, MULT, ADD)
                nc.vector.tensor_tensor(OT[:, 0:h], t1, V[:], MULT)
                # t0 written straight into the out slab by ACT
                nc.scalar.activation(
                    OT[:, h:2 * h], W[:, 0:h], SIN, bias=biases[0].ap(), scale=PI
                )
                nc.sync.dma_start(youts[t][:], OT[:])

            pre(0)
            if nt > 1:
                pre(1)
            for t in range(nt):
                post(t)
                if t + 2 < nt:
                    pre(t + 2)

    nc.compile()
    return nc


_PROGRAM_CACHE = {}


def _get_program(consts, nt=NT, h=H):
    key = (tuple(float(np.float32(v)) for v in consts[:3]), nt, h)
    if key not in _PROGRAM_CACHE:
        _PROGRAM_CACHE[key] = build_program(consts, nt, h)
    return _PROGRAM_CACHE[key]


def make_in_maps(inputs, nt=NT, h=H, n_cores=N_CORES):
    """Shard full inputs into per-core fp16 input maps (host de-interleave)."""
    x = np.asarray(inputs)
    xh = x.astype(np.float16)
    x0 = xh[:, 0].reshape(n_cores, nt, P, h)
    x1 = xh[:, 1].reshape(n_cores, nt, P, h)
    xin = np.concatenate([x0, x1], axis=-1)  # [cores, nt, P, 2h]
    return [{"xin": xin[i]} for i in range(n_cores)]


def kernel(inputs, weights):
    """Full inputs in, full output out (see module docstring)."""
    consts = _derive_consts(weights)
    nc = _get_program(consts)
    in_maps = make_in_maps(inputs)
    res = run_bass_kernel_spmd(nc, in_maps, list(range(N_CORES)))
    p, case_b, S = consts[3], consts[4], consts[5]
    cm2, ct0 = (1.0, p) if case_b else (p, 1.0)
    chunks = []
    for r in res.results:
        for t in range(NT):
            y = np.asarray(r[f"y{t}"], dtype=np.float32)  # [P, 2H]: [M2 | t0]
            chunks.append(
                np.float32(cm2) * y[:, 0:H].reshape(-1)
                + np.float32(ct0) * y[:, H:2 * H].reshape(-1)
            )
    out = np.float32(S) * np.concatenate(chunks)
    return out.reshape(B, 1).astype(np.float32)
